# revision 3
# baseline (speedup 1.0000x reference)
"""TRN2 Bass kernel for nn_CrossModalAttentionFusion (8-core data parallel).

Mathematical structure of the reference module:
    e  = LN(efficient_features);  x = LN(xception_features)
    Q,K,V = per-head projections; scores over a seq_len==1 key axis
    softmax(scores) over a singleton axis == 1.0 exactly  =>  head_out = V
    out = LN3(V_flat @ Wo + bo)

Hence the module collapses to:
    out = LN3( LN2(x) @ (Wv_flat @ Wo) + (bv_flat @ Wo + bo) )

which we further fold (host-side, input-independent weight preprocessing):
    W_eff = diag(g2) @ Wv_flat @ Wo
    W'    = W_eff - (1/XC) * ones ⊗ colsum(W_eff)     (folds LN2 mean-subtract)
    b_c   = b2 @ Wv_flat @ Wo + bv_flat @ Wo + bo      (folds LN2 shift)
    sig   = sqrt(var(x_row) + EPS)                     (per row)
    z     = x @ W' + sig * b_c                         (folds LN2 scale into LN3
                                                        via scale-invariance)
    out   = (z - mean(z)) / sqrt(var(z) + EPS*sig^2) * g3 + b3

Device work per row: one 2048x1024 matmul (float32r on the PE at bf16 speed),
row stats via bn_stats/bn_aggr, and a fused epilogue.
Data parallel: batch 16384 split as 2048 rows per NeuronCore, weights
replicated; no collectives — outputs are gathered on host.
"""

from contextlib import ExitStack

import numpy as np
import ml_dtypes

import concourse.bacc as bacc
import concourse.bass as bass
import concourse.mybir as mybir
import concourse.tile as tile
from concourse.bass_utils import run_bass_kernel_spmd

# Problem constants (hardcoded per spec)
N_CORES = 8
BATCH = 16384
XC = 2048           # xception feature dim (contraction)
D_OUT = 1024        # output dim
EPS = 1e-5

P = 128             # partitions
B_LOC = BATCH // N_CORES          # rows per core (2048)
KT = XC // P                      # contraction k-tiles (16)
B_BLK = 512                       # rows per DMA block
NB = B_LOC // B_BLK               # blocks per core (4)
BT = B_BLK // P                   # 128-row tiles per block (4)
NT = B_LOC // P                   # total 128-row tiles per core (16)

f32 = mybir.dt.float32
f32r = mybir.dt.float32r
bf16 = mybir.dt.bfloat16
Act = mybir.ActivationFunctionType
Alu = mybir.AluOpType

_NC_CACHE = {}


def build_nc(apply_g3b3: bool):
    """Build the per-core SPMD Bass program (same program on all 8 cores)."""
    nc = bacc.Bacc(name="cmaf")
    # DRAM I/O (per core)
    xt = nc.declare_dram_parameter("xt", [NB, KT, P, B_BLK], f32r, isOutput=False)
    xn = nc.declare_dram_parameter("xn", [NT, P, XC], bf16, isOutput=False)
    w = nc.declare_dram_parameter("w", [KT, P, D_OUT], f32r, isOutput=False)
    beff = nc.declare_dram_parameter("beff", [P, D_OUT], f32, isOutput=False)
    if apply_g3b3:
        g3r = nc.declare_dram_parameter("g3r", [P, D_OUT], f32, isOutput=False)
        b3r = nc.declare_dram_parameter("b3r", [P, D_OUT], f32, isOutput=False)
    out = nc.declare_dram_parameter("out", [B_LOC, D_OUT], f32, isOutput=True)

    with ExitStack() as ctx:
        tc = ctx.enter_context(tile.TileContext(nc))
        wpool = ctx.enter_context(tc.tile_pool(name="w", bufs=1))
        cpool = ctx.enter_context(tc.tile_pool(name="c", bufs=1))
        xpool = ctx.enter_context(tc.tile_pool(name="x", bufs=2))
        npool = ctx.enter_context(tc.tile_pool(name="n", bufs=3))
        zpool = ctx.enter_context(tc.tile_pool(name="z", bufs=2))
        opool = ctx.enter_context(tc.tile_pool(name="o", bufs=3))
        spool = ctx.enter_context(tc.tile_pool(name="s", bufs=4))
        psum = ctx.enter_context(tc.tile_pool(name="ps", bufs=2, space="PSUM"))

        # Resident weights: 16 k-tiles of W' [128, 1024]
        w_t = []
        for kt in range(KT):
            wt = wpool.tile([P, D_OUT], f32r, tag=f"w{kt}")
            nc.sync.dma_start(out=wt[:], in_=w[kt])
            w_t.append(wt)
        beff_sb = cpool.tile([P, D_OUT], f32, tag="beff")
        nc.sync.dma_start(out=beff_sb[:], in_=beff[:])
        eps_sb = cpool.tile([P, 1], f32, tag="eps")
        nc.vector.memset(eps_sb[:], EPS)
        if apply_g3b3:
            g3_sb = cpool.tile([P, D_OUT], f32, tag="g3")
            b3_sb = cpool.tile([P, D_OUT], f32, tag="b3")
            nc.sync.dma_start(out=g3_sb[:], in_=g3r[:])
            nc.sync.dma_start(out=b3_sb[:], in_=b3r[:])

        for blk in range(NB):
            # Load the transposed activation block: 16 tiles [128k, 512b]
            xt_t = []
            for kt in range(KT):
                xtt = xpool.tile([P, B_BLK], f32r, tag=f"xt{kt}")
                nc.sync.dma_start(out=xtt[:], in_=xt[blk, kt])
                xt_t.append(xtt)

            for bt in range(BT):
                g = blk * BT + bt
                # Natural-layout tile for row stats
                xn_sb = npool.tile([P, XC], bf16, tag="xn")
                nc.sync.dma_start(out=xn_sb[:], in_=xn[g])

                # Row stats of x: mean/var over the 2048 free elements
                st = spool.tile([P, XC // 512, 6], f32, tag="st")
                for cch in range(XC // 512):
                    nc.vector.bn_stats(st[:, cch, :], xn_sb[:, bass.ts(cch, 512)])
                agg = spool.tile([P, 2], f32, tag="agg")
                nc.vector.bn_aggr(agg[:], st[:])
                # sig = sqrt(var + EPS)
                sig = spool.tile([P, 1], f32, tag="sig")
                nc.scalar.activation(sig[:], agg[:, 1:2], Act.Sqrt,
                                     bias=eps_sb[:], scale=1.0)

                # y[b, :] = x_row @ W'  accumulated over 16 k-tiles
                ps0 = psum.tile([P, 512], f32, tag="ps0")
                ps1 = psum.tile([P, 512], f32, tag="ps1")
                for kt in range(KT):
                    lhsT = xt_t[kt][:, bass.ts(bt, P)]
                    nc.tensor.matmul(
                        ps0[:], lhsT=lhsT, rhs=w_t[kt][:, 0:512],
                        start=(kt == 0), stop=(kt == KT - 1),
                    )
                    nc.tensor.matmul(
                        ps1[:], lhsT=lhsT, rhs=w_t[kt][:, 512:1024],
                        start=(kt == 0), stop=(kt == KT - 1),
                    )

                # z = y + sig * b_c
                z = zpool.tile([P, D_OUT], f32, tag="z")
                nc.vector.scalar_tensor_tensor(
                    z[:, 0:512], beff_sb[:, 0:512], sig[:], ps0[:],
                    op0=Alu.mult, op1=Alu.add,
                )
                nc.vector.scalar_tensor_tensor(
                    z[:, 512:1024], beff_sb[:, 512:1024], sig[:], ps1[:],
                    op0=Alu.mult, op1=Alu.add,
                )

                # LN3 stats over z
                st3 = spool.tile([P, 2, 6], f32, tag="st3")
                nc.vector.bn_stats(st3[:, 0, :], z[:, 0:512])
                nc.vector.bn_stats(st3[:, 1, :], z[:, 512:1024])
                agg3 = spool.tile([P, 2], f32, tag="agg3")
                nc.vector.bn_aggr(agg3[:], st3[:])

                # r = 1/sqrt(var3 + EPS*sig^2);  nb = -mean3 * r
                sig2 = spool.tile([P, 1], f32, tag="sig2")
                nc.vector.tensor_mul(sig2[:], sig[:], sig[:])
                t3 = spool.tile([P, 1], f32, tag="t3")
                nc.vector.scalar_tensor_tensor(
                    t3[:], sig2[:], float(EPS), agg3[:, 1:2],
                    op0=Alu.mult, op1=Alu.add,
                )
                rt = spool.tile([P, 1], f32, tag="rt")
                nc.scalar.activation(rt[:], t3[:], Act.Sqrt, bias=0.0, scale=1.0)
                r3 = spool.tile([P, 1], f32, tag="r3")
                nc.vector.reciprocal(r3[:], rt[:])
                nb3 = spool.tile([P, 1], f32, tag="nb3")
                nc.vector.scalar_tensor_tensor(
                    nb3[:], agg3[:, 0:1], -1.0, r3[:],
                    op0=Alu.mult, op1=Alu.mult,
                )

                # out = z*r + nb  (per-partition scale/bias on the ACT engine)
                ot = opool.tile([P, D_OUT], f32, tag="ot")
                nc.scalar.activation(ot[:], z[:], Act.Identity, bias=nb3[:], scale=r3[:])
                if apply_g3b3:
                    nc.vector.tensor_mul(ot[:], ot[:], g3_sb[:])
                    nc.vector.tensor_add(ot[:], ot[:], b3_sb[:])
                nc.sync.dma_start(out=out[bass.ts(g, P), :], in_=ot[:])

    nc.finalize()
    return nc


def get_nc(apply_g3b3: bool):
    key = bool(apply_g3b3)
    if key not in _NC_CACHE:
        _NC_CACHE[key] = build_nc(key)
    return _NC_CACHE[key]


def fold_weights(Wv, bv, Wo, bo, g2, b2):
    """Host-side, input-independent weight folding (float64 for accuracy)."""
    Wv_flat = np.transpose(np.asarray(Wv, np.float64), (1, 0, 2)).reshape(XC, D_OUT)
    Wo64 = np.asarray(Wo, np.float64)
    W_eff = (np.asarray(g2, np.float64)[:, None] * Wv_flat) @ Wo64
    c = W_eff.sum(axis=0)
    Wp = (W_eff - c[None, :] / XC).astype(np.float32)
    b_c = (
        np.asarray(b2, np.float64) @ Wv_flat @ Wo64
        + np.asarray(bv, np.float64).reshape(-1) @ Wo64
        + np.asarray(bo, np.float64)
    ).astype(np.float32)
    return Wp, b_c


def prepare_in_maps(xception_features, Wp, b_c, g3=None, b3=None):
    """Shard the batch across 8 cores; replicate weights."""
    x = np.asarray(xception_features, np.float32)
    w_tiled = np.ascontiguousarray(Wp.reshape(KT, P, D_OUT))
    beff_rep = np.ascontiguousarray(np.broadcast_to(b_c, (P, D_OUT)))
    extra = {}
    if g3 is not None:
        extra["g3r"] = np.ascontiguousarray(
            np.broadcast_to(np.asarray(g3, np.float32), (P, D_OUT)))
        extra["b3r"] = np.ascontiguousarray(
            np.broadcast_to(np.asarray(b3, np.float32), (P, D_OUT)))
    in_maps = []
    for cidx in range(N_CORES):
        xs = x[cidx * B_LOC:(cidx + 1) * B_LOC]  # [2048, 2048]
        # xt[blk, kt, p, j] = xs[blk*512 + j, kt*128 + p]
        xt_b = np.ascontiguousarray(
            xs.reshape(NB, B_BLK, KT, P).transpose(0, 2, 3, 1))
        xn_b = np.ascontiguousarray(
            xs.astype(ml_dtypes.bfloat16).reshape(NT, P, XC))
        in_maps.append(
            {"xt": xt_b, "xn": xn_b, "w": w_tiled, "beff": beff_rep, **extra})
    return in_maps


def kernel(efficient_features, xception_features, Wq, bq, Wk, bk, Wv, bv,
           Wo, bo, g1, b1, g2, b2, g3, b3):
    # softmax over the singleton key axis == 1 exactly, so Q/K branches and
    # efficient_features/g1/b1 cannot affect the output.
    Wp, b_c = fold_weights(Wv, bv, Wo, bo, g2, b2)
    g3a = np.asarray(g3, np.float32)
    b3a = np.asarray(b3, np.float32)
    trivial = bool(np.all(g3a == 1.0) and np.all(b3a == 0.0))
    nc = get_nc(apply_g3b3=not trivial)
    in_maps = prepare_in_maps(
        xception_features, Wp, b_c,
        g3=None if trivial else g3a, b3=None if trivial else b3a)
    res = run_bass_kernel_spmd(nc, in_maps, core_ids=list(range(N_CORES)))
    return np.concatenate([res.results[c]["out"] for c in range(N_CORES)], axis=0)


# revision 8
# speedup vs baseline: 700.4416x; 700.4416x over previous
"""TRN2 Bass kernel for nn_CrossModalAttentionFusion (8-core data parallel).

Mathematical structure of the reference module:
    e  = LN(efficient_features);  x = LN(xception_features)
    Q,K,V = per-head projections; scores over a seq_len==1 key axis
    softmax(scores) over a singleton axis == 1.0 exactly  =>  head_out = V
    out = LN3(V_flat @ Wo + bo)

Hence the module collapses to:
    out = LN3( LN2(x) @ (Wv_flat @ Wo) + (bv_flat @ Wo + bo) )

which we further fold (host-side, input-independent weight preprocessing):
    W_eff = diag(g2) @ Wv_flat @ Wo
    W'    = W_eff - (1/XC) * ones ⊗ colsum(W_eff)     (folds LN2 mean-subtract)
    b_c   = b2 @ Wv_flat @ Wo + bv_flat @ Wo + bo      (folds LN2 shift)
    sig   = sqrt(var(x_row) + EPS)                     (per row)
    z     = x @ W' + sig * b_c                         (folds LN2 scale into LN3
                                                        via scale-invariance)
    out   = (z - mean(z)) / sqrt(var(z) + EPS*sig^2) * g3 + b3

Device work per row: one 2048x1024 matmul (float32r on the PE at bf16 speed),
row stats via bn_stats/bn_aggr, and a fused epilogue.
Data parallel: batch 16384 split as 2048 rows per NeuronCore, weights
replicated; no collectives — outputs are gathered on host.
"""

from contextlib import ExitStack

import numpy as np
import ml_dtypes

import concourse.bacc as bacc
import concourse.bass as bass
import concourse.mybir as mybir
import concourse.tile as tile
from concourse.bass_utils import run_bass_kernel_spmd

# Problem constants (hardcoded per spec)
N_CORES = 8
BATCH = 16384
XC = 2048           # xception feature dim (contraction)
D_OUT = 1024        # output dim
EPS = 1e-5

P = 128             # partitions
B_LOC = BATCH // N_CORES          # rows per core (2048)
KT = XC // P                      # contraction k-tiles (16)
B_BLK = 512                       # rows per DMA block
NB = B_LOC // B_BLK               # blocks per core (4)
BT = B_BLK // P                   # 128-row tiles per block (4)
NT = B_LOC // P                   # total 128-row tiles per core (16)

f32 = mybir.dt.float32
f32r = mybir.dt.float32r
bf16 = mybir.dt.bfloat16
Act = mybir.ActivationFunctionType
Alu = mybir.AluOpType

_NC_CACHE = {}


def build_nc(apply_g3b3: bool, reps: int = 1):
    """Build the per-core SPMD Bass program (same program on all 8 cores).

    reps>1 unrolls the whole kernel body (including weight loads) that many
    times — used only for timing (marginal per-rep cost ~= steady-state
    kernel time, free of the ~80ms fixed per-call axon dispatch overhead).
    """
    nc = bacc.Bacc(name="cmaf")
    # DRAM I/O (per core)
    xt = nc.declare_dram_parameter("xt", [NB, KT, P, B_BLK], f32r, isOutput=False)
    xn = nc.declare_dram_parameter("xn", [NT, P, XC], bf16, isOutput=False)
    w = nc.declare_dram_parameter("w", [KT, P, D_OUT], f32r, isOutput=False)
    beff = nc.declare_dram_parameter("beff", [P, D_OUT], f32, isOutput=False)
    if apply_g3b3:
        g3r = nc.declare_dram_parameter("g3r", [P, D_OUT], f32, isOutput=False)
        b3r = nc.declare_dram_parameter("b3r", [P, D_OUT], f32, isOutput=False)
    out = nc.declare_dram_parameter("out", [B_LOC, D_OUT], f32, isOutput=True)

    with ExitStack() as ctx:
        tc = ctx.enter_context(tile.TileContext(nc))
        wpool = ctx.enter_context(tc.tile_pool(name="w", bufs=1))
        cpool = ctx.enter_context(tc.tile_pool(name="c", bufs=1))
        xpool = ctx.enter_context(tc.tile_pool(name="x", bufs=2))
        npool = ctx.enter_context(tc.tile_pool(name="n", bufs=3))
        zpool = ctx.enter_context(tc.tile_pool(name="z", bufs=2))
        opool = ctx.enter_context(tc.tile_pool(name="o", bufs=3))
        spool = ctx.enter_context(tc.tile_pool(name="s", bufs=4))
        psum = ctx.enter_context(tc.tile_pool(name="ps", bufs=2, space="PSUM"))

        # Resident weights: 16 k-tiles of W' [128, 1024]
        w_t = []
        for kt in range(KT):
            wt = wpool.tile([P, D_OUT], f32r, tag=f"w{kt}")
            nc.sync.dma_start(out=wt[:], in_=w[kt])
            w_t.append(wt)
        beff_sb = cpool.tile([P, D_OUT], f32, tag="beff")
        nc.sync.dma_start(out=beff_sb[:], in_=beff[:])
        eps_sb = cpool.tile([P, 1], f32, tag="eps")
        nc.vector.memset(eps_sb[:], EPS)
        if apply_g3b3:
            g3_sb = cpool.tile([P, D_OUT], f32, tag="g3")
            b3_sb = cpool.tile([P, D_OUT], f32, tag="b3")
            nc.sync.dma_start(out=g3_sb[:], in_=g3r[:])
            nc.sync.dma_start(out=b3_sb[:], in_=b3r[:])

        for rep in range(reps):
          for blk in range(NB):
            # Load the transposed activation block: 16 tiles [128k, 512b]
            xt_t = []
            for kt in range(KT):
                xtt = xpool.tile([P, B_BLK], f32r, tag=f"xt{kt}")
                nc.sync.dma_start(out=xtt[:], in_=xt[blk, kt])
                xt_t.append(xtt)

            for bt in range(BT):
                g = blk * BT + bt
                # Natural-layout tile for row stats
                xn_sb = npool.tile([P, XC], bf16, tag="xn")
                nc.sync.dma_start(out=xn_sb[:], in_=xn[g])

                # Row stats of x: mean/var over the 2048 free elements
                st = spool.tile([P, XC // 512, 6], f32, tag="st")
                for cch in range(XC // 512):
                    nc.vector.bn_stats(st[:, cch, :], xn_sb[:, bass.ts(cch, 512)])
                agg = spool.tile([P, 2], f32, tag="agg")
                nc.vector.bn_aggr(agg[:], st[:])
                # sig = sqrt(var + EPS)
                sig = spool.tile([P, 1], f32, tag="sig")
                nc.scalar.activation(sig[:], agg[:, 1:2], Act.Sqrt,
                                     bias=eps_sb[:], scale=1.0)

                # y[b, :] = x_row @ W'  accumulated over 16 k-tiles
                ps0 = psum.tile([P, 512], f32, tag="ps0")
                ps1 = psum.tile([P, 512], f32, tag="ps1")
                for kt in range(KT):
                    lhsT = xt_t[kt][:, bass.ts(bt, P)]
                    nc.tensor.matmul(
                        ps0[:], lhsT=lhsT, rhs=w_t[kt][:, 0:512],
                        start=(kt == 0), stop=(kt == KT - 1),
                    )
                    nc.tensor.matmul(
                        ps1[:], lhsT=lhsT, rhs=w_t[kt][:, 512:1024],
                        start=(kt == 0), stop=(kt == KT - 1),
                    )

                # z = y + sig * b_c
                z = zpool.tile([P, D_OUT], f32, tag="z")
                nc.vector.scalar_tensor_tensor(
                    z[:, 0:512], beff_sb[:, 0:512], sig[:], ps0[:],
                    op0=Alu.mult, op1=Alu.add,
                )
                nc.vector.scalar_tensor_tensor(
                    z[:, 512:1024], beff_sb[:, 512:1024], sig[:], ps1[:],
                    op0=Alu.mult, op1=Alu.add,
                )

                # LN3 stats over z
                st3 = spool.tile([P, 2, 6], f32, tag="st3")
                nc.vector.bn_stats(st3[:, 0, :], z[:, 0:512])
                nc.vector.bn_stats(st3[:, 1, :], z[:, 512:1024])
                agg3 = spool.tile([P, 2], f32, tag="agg3")
                nc.vector.bn_aggr(agg3[:], st3[:])

                # r = 1/sqrt(var3 + EPS*sig^2);  nb = -mean3 * r
                sig2 = spool.tile([P, 1], f32, tag="sig2")
                nc.vector.tensor_mul(sig2[:], sig[:], sig[:])
                t3 = spool.tile([P, 1], f32, tag="t3")
                nc.vector.scalar_tensor_tensor(
                    t3[:], sig2[:], float(EPS), agg3[:, 1:2],
                    op0=Alu.mult, op1=Alu.add,
                )
                rt = spool.tile([P, 1], f32, tag="rt")
                nc.scalar.activation(rt[:], t3[:], Act.Sqrt, bias=0.0, scale=1.0)
                r3 = spool.tile([P, 1], f32, tag="r3")
                nc.vector.reciprocal(r3[:], rt[:])
                nb3 = spool.tile([P, 1], f32, tag="nb3")
                nc.vector.scalar_tensor_tensor(
                    nb3[:], agg3[:, 0:1], -1.0, r3[:],
                    op0=Alu.mult, op1=Alu.mult,
                )

                # out = z*r + nb  (per-partition scale/bias on the ACT engine)
                ot = opool.tile([P, D_OUT], f32, tag="ot")
                nc.scalar.activation(ot[:], z[:], Act.Identity, bias=nb3[:], scale=r3[:])
                if apply_g3b3:
                    nc.vector.tensor_mul(ot[:], ot[:], g3_sb[:])
                    nc.vector.tensor_add(ot[:], ot[:], b3_sb[:])
                nc.sync.dma_start(out=out[bass.ts(g, P), :], in_=ot[:])

    nc.finalize()
    return nc


def get_nc(apply_g3b3: bool, reps: int = 1):
    key = (bool(apply_g3b3), reps)
    if key not in _NC_CACHE:
        _NC_CACHE[key] = build_nc(apply_g3b3, reps)
    return _NC_CACHE[key]


def fold_weights(Wv, bv, Wo, bo, g2, b2):
    """Host-side, input-independent weight folding (float64 for accuracy)."""
    Wv_flat = np.transpose(np.asarray(Wv, np.float64), (1, 0, 2)).reshape(XC, D_OUT)
    Wo64 = np.asarray(Wo, np.float64)
    W_eff = (np.asarray(g2, np.float64)[:, None] * Wv_flat) @ Wo64
    c = W_eff.sum(axis=0)
    Wp = (W_eff - c[None, :] / XC).astype(np.float32)
    b_c = (
        np.asarray(b2, np.float64) @ Wv_flat @ Wo64
        + np.asarray(bv, np.float64).reshape(-1) @ Wo64
        + np.asarray(bo, np.float64)
    ).astype(np.float32)
    return Wp, b_c


def prepare_in_maps(xception_features, Wp, b_c, g3=None, b3=None):
    """Shard the batch across 8 cores; replicate weights."""
    x = np.asarray(xception_features, np.float32)
    w_tiled = np.ascontiguousarray(Wp.reshape(KT, P, D_OUT))
    beff_rep = np.ascontiguousarray(np.broadcast_to(b_c, (P, D_OUT)))
    extra = {}
    if g3 is not None:
        extra["g3r"] = np.ascontiguousarray(
            np.broadcast_to(np.asarray(g3, np.float32), (P, D_OUT)))
        extra["b3r"] = np.ascontiguousarray(
            np.broadcast_to(np.asarray(b3, np.float32), (P, D_OUT)))
    in_maps = []
    for cidx in range(N_CORES):
        xs = x[cidx * B_LOC:(cidx + 1) * B_LOC]  # [2048, 2048]
        # xt[blk, kt, p, j] = xs[blk*512 + j, kt*128 + p]
        xt_b = np.ascontiguousarray(
            xs.reshape(NB, B_BLK, KT, P).transpose(0, 2, 3, 1))
        xn_b = np.ascontiguousarray(
            xs.astype(ml_dtypes.bfloat16).reshape(NT, P, XC))
        in_maps.append(
            {"xt": xt_b, "xn": xn_b, "w": w_tiled, "beff": beff_rep, **extra})
    return in_maps


def kernel(efficient_features, xception_features, Wq, bq, Wk, bk, Wv, bv,
           Wo, bo, g1, b1, g2, b2, g3, b3):
    # softmax over the singleton key axis == 1 exactly, so Q/K branches and
    # efficient_features/g1/b1 cannot affect the output.
    Wp, b_c = fold_weights(Wv, bv, Wo, bo, g2, b2)
    g3a = np.asarray(g3, np.float32)
    b3a = np.asarray(b3, np.float32)
    trivial = bool(np.all(g3a == 1.0) and np.all(b3a == 0.0))
    nc = get_nc(apply_g3b3=not trivial)
    in_maps = prepare_in_maps(
        xception_features, Wp, b_c,
        g3=None if trivial else g3a, b3=None if trivial else b3a)
    res = run_bass_kernel_spmd(nc, in_maps, core_ids=list(range(N_CORES)))
    return np.concatenate([res.results[c]["out"] for c in range(N_CORES)], axis=0)
